# revision 29
# baseline (speedup 1.0000x reference)
"""DockingScorePredictor Trainium2 kernel.

Only pairs inside the 8A cutoff matter (~43%), so the host compacts valid
pairs into dense 512-pair tiles.  The per-pair layer-1 preactivation
  z1 = W1a.T hp + W1b.T hl + b1 + W1c.T rb(d)
depends on atom *types* (20/16), so z1 collapses to a single K=68 matmul
against [rb(32); onehot_ptype(20); onehot_ltype(16)] with a fused weight
[W1c; prot_emb@W1a; lig_emb@W1b + b1].

The ~230 tiles of all 8 complexes are bin-packed across the 8 cores (29 per
core); per-tile channel sums (acc) are attributed back to complexes on the
host.  Device work per tile: 3 fp16 matmuls (K=68 z1, W2, W3; N=512, 1
col/cycle) + 3 relu evacuations, software-pipelined 8 steps deep with
2-step slack on the PSUM->engine edges.  relu3+accum always runs on DVE
(its accumulator read is ~9ns vs ACT's ~182ns, and pad tiles then round
identically to the extracted cvec); relu1 is split 40/60 DVE/ACT so DVE
stays the pacer.  The first DMA carries W1z plus rhs tile 0 so one
transfer gates the first matmul.

Pad columns (zero rhs) contribute relu(W3.T relu(b2) + b3); that constant
(cvec) is computed early on-device via an exact 1-column replica of the
tile pipeline, and the host subtracts npad*cvec per complex, then runs the
tiny fp32 scoring head while unsharding.  acc ships as two separate output
tensors so the bulk's DMA retires while the pipeline drains.
"""
import numpy as np
from contextlib import ExitStack

import concourse.bass as bass
import concourse.bacc as bacc
import concourse.tile as tile
from concourse import mybir
from concourse import bass_utils

F32 = mybir.dt.float32
FP16 = mybir.dt.float16
AF = mybir.ActivationFunctionType
ALU = mybir.AluOpType

B, P, L = 8, 512, 64
H, RB = 128, 32
NPT, NLT = 20, 16
CUTOFF = 8.0
N_CORES = 8
K1 = RB + NPT + NLT          # 68: contraction dim of the fused z1 matmul
NT_DEFAULT = 29              # tiles of 512 pairs per core
WIDTH = 0.5 * CUTOFF / RB + 1e-8

_CACHE = {}


def _build_nc(nt):
    nc = bacc.Bacc("TRN2", target_bir_lowering=False, debug=False,
                   num_devices=N_CORES)
    d = {}

    def inp(name, shape, dt):
        d[name] = nc.dram_tensor(name, shape, dt, kind="ExternalInput").ap()

    inp("rhsG", [K1, nt * 512], FP16)
    inp("wa", [K1, H + 512], FP16)  # W1z | rhs tile 0
    inp("wb", [H, 2 * H], FP16)   # W2 | W3
    inp("fb", [H, 2], F32)        # b2 | b3

    accA_ap = nc.dram_tensor("accA", [H, nt - 5], F32,
                             kind="ExternalOutput").ap()
    accB_ap = nc.dram_tensor("accB", [H, 5], F32, kind="ExternalOutput").ap()
    cvec_ap = nc.dram_tensor("cvec", [H, 1], F32, kind="ExternalOutput").ap()

    with tile.TileContext(nc) as tc:
        with ExitStack() as ctx:
            const = ctx.enter_context(tc.tile_pool(name="const", bufs=1))
            rbuf = ctx.enter_context(tc.tile_pool(name="rbuf", bufs=6))
            abuf = ctx.enter_context(tc.tile_pool(name="abuf", bufs=3))
            psZ1 = ctx.enter_context(tc.tile_pool(name="psZ1", bufs=3, space="PSUM"))
            psZ2 = ctx.enter_context(tc.tile_pool(name="psZ2", bufs=3, space="PSUM"))
            psZ3 = ctx.enter_context(tc.tile_pool(name="psZ3", bufs=2, space="PSUM"))

            rtiles, z1s, a1s, z2s, a2s, z3s = {}, {}, {}, {}, {}, {}
            PREF = 5

            def dma_rhs(g):
                rt = rbuf.tile([K1, 512], FP16, tag="rhs", name=f"rhs{g}")
                nc.sync.dma_start(out=rt[:, :],
                                  in_=d["rhsG"][:, 512 * g:512 * (g + 1)])
                rtiles[g] = rt

            # one combined first DMA (W1z weights + rhs tile 0) gates MM0
            wa = const.tile([K1, H + 512], FP16, tag="wa", name="wa")
            nc.sync.dma_start(out=wa, in_=d["wa"])
            wb = const.tile([H, 2 * H], FP16, tag="wb", name="wb")
            nc.sync.dma_start(out=wb, in_=d["wb"])
            fb = const.tile([H, 2], F32, tag="fb", name="fb")
            nc.sync.dma_start(out=fb, in_=d["fb"])
            rtiles[0] = wa[:, H:H + 512]
            for g in range(1, PREF):
                dma_rhs(g)
            W1z = wa[:, 0:H]
            W2 = wb[:, 0:H]
            W3 = wb[:, H:2 * H]

            # warm the ACT table set before the pipeline needs relu
            warm = const.tile([1, 64], F32, tag="warm", name="warm")
            nc.vector.memset(warm[:, :], 0.0)
            nc.scalar.activation(out=warm[:, :], in_=warm[:, :], func=AF.Relu,
                                 bias=0.0, scale=1.0)
            # biases as dense [H,1] tiles
            b2 = const.tile([H, 1], F32, tag="b2", name="b2")
            nc.vector.tensor_scalar(out=b2[:, :], in0=fb[:, 0:1],
                                    scalar1=0.0, scalar2=None, op0=ALU.add)
            b3 = const.tile([H, 1], F32, tag="b3", name="b3")
            nc.vector.tensor_scalar(out=b3[:, :], in0=fb[:, 1:2],
                                    scalar1=0.0, scalar2=None, op0=ALU.add)

            acc = const.tile([H, nt], F32, tag="acc", name="acc")
            zeros = const.tile([H, 512], F32, tag="zeros", name="zeros")
            nc.vector.memset(zeros[:, :], 0.0)

            # cvec (the constant a3 of an all-zero pad column) via an exact
            # 1-column replica of the tile pipeline, shipped early instead of
            # end-gating the kernel on the last tile's activation
            a2c = const.tile([H, 1], FP16, tag="a2c", name="a2c")
            nc.scalar.activation(out=a2c[:, :], in_=zeros[:, 0:1],
                                 func=AF.Relu, bias=b2, scale=1.0)
            z3c = psZ3.tile([H, 1], F32, tag="z3", name="z3c")
            nc.tensor.matmul(out=z3c[:, :], lhsT=W3, rhs=a2c[:, :],
                             start=True, stop=True)
            cvec_t = const.tile([H, 1], F32, tag="cvec", name="cvec_t")
            nc.vector.scalar_tensor_tensor(out=cvec_t[:, :], in0=z3c[:, :],
                                           scalar=b3, in1=zeros[:, 0:1],
                                           op0=ALU.add, op1=ALU.max)
            nc.sync.dma_start(out=cvec_ap, in_=cvec_t[:, :])
            for step in range(nt + 8):
                t0 = step
                if t0 < nt:
                    if t0 + PREF < nt:
                        dma_rhs(t0 + PREF)
                    z1 = psZ1.tile([H, 512], F32, tag="z1", name=f"z1_{t0}")
                    z1s[t0] = z1
                    rt = rtiles.pop(t0)
                    rhs_ap = rt if t0 == 0 else rt[:, :]
                    nc.tensor.matmul(out=z1[:, :], lhsT=W1z,
                                     rhs=rhs_ap,
                                     start=True, stop=True)
                t1 = step - 2
                if 0 <= t1 < nt:
                    a1 = abuf.tile([H, 512], FP16, tag="a1", name=f"a1_{t1}")
                    a1s[t1] = a1
                    if t1 % 5 < 2:
                        nc.vector.tensor_scalar(out=a1[:, :],
                                                in0=z1s.pop(t1)[:, :],
                                                scalar1=0.0, scalar2=None,
                                                op0=ALU.max)
                    else:
                        nc.scalar.activation(out=a1[:, :],
                                             in_=z1s.pop(t1)[:, :],
                                             func=AF.Relu, bias=0.0,
                                             scale=1.0)
                t2 = step - 3
                if 0 <= t2 < nt:
                    z2 = psZ2.tile([H, 512], F32, tag="z2", name=f"z2_{t2}")
                    z2s[t2] = z2
                    nc.tensor.matmul(out=z2[:, :], lhsT=W2,
                                     rhs=a1s.pop(t2)[:, :],
                                     start=True, stop=True)
                t3 = step - 5
                if 0 <= t3 < nt:
                    a2 = abuf.tile([H, 512], FP16, tag="a2", name=f"a2_{t3}")
                    a2s[t3] = a2
                    nc.scalar.activation(out=a2[:, :], in_=z2s.pop(t3)[:, :],
                                         func=AF.Relu, bias=b2, scale=1.0)
                t4 = step - 6
                if 0 <= t4 < nt:
                    z3 = psZ3.tile([H, 512], F32, tag="z3", name=f"z3_{t4}")
                    z3s[t4] = z3
                    nc.tensor.matmul(out=z3[:, :], lhsT=W3,
                                     rhs=a2s.pop(t4)[:, :],
                                     start=True, stop=True)
                t5 = step - 7
                if 0 <= t5 < nt:
                    a3 = abuf.tile([H, 512], FP16, tag="a3", name=f"a3_{t5}")
                    # out = max(z3 + b3, 0); accum = sum(out).  Always DVE:
                    # its accumulator read is ~9ns vs ACT's ~182ns, and all
                    # pad tiles round identically to the extracted cvec.
                    nc.vector.scalar_tensor_tensor(
                        out=a3[:, :], in0=z3s.pop(t5)[:, :],
                        scalar=b3, in1=zeros[:, :],
                        op0=ALU.add, op1=ALU.max,
                        accum_out=acc[:, t5:t5 + 1])
            nc.sync.dma_start(out=accA_ap, in_=acc[:, 0:nt - 5])
            nc.sync.dma_start(out=accB_ap, in_=acc[:, nt - 5:nt])

    nc.compile()
    return nc


def _get_nc(nt=NT_DEFAULT):
    if nt not in _CACHE:
        _CACHE[nt] = _build_nc(nt)
    return _CACHE[nt]


def kernel(protein_pos, ligand_pos, prot_emb, lig_emb,
           W1, b1, W2, b2, W3, b3, Wr1, br1, Wr2, br2,
           protein_atom_type, ligand_atom_type, protein_batch, ligand_batch):
    protein_pos = np.asarray(protein_pos, dtype=np.float32).reshape(B, P, 3)
    ligand_pos = np.asarray(ligand_pos, dtype=np.float32).reshape(B, L, 3)
    prot_emb = np.asarray(prot_emb, dtype=np.float32)
    lig_emb = np.asarray(lig_emb, dtype=np.float32)
    W1 = np.asarray(W1, dtype=np.float32)
    b1 = np.asarray(b1, dtype=np.float32)
    W2 = np.asarray(W2, dtype=np.float32)
    W3 = np.asarray(W3, dtype=np.float32)
    Wr1 = np.asarray(Wr1, dtype=np.float32)
    br1 = np.asarray(br1, dtype=np.float32).reshape(H)
    Wr2 = np.asarray(Wr2, dtype=np.float32).reshape(H)
    br2 = float(np.asarray(br2, dtype=np.float32).reshape(()))
    ptype = np.asarray(protein_atom_type).reshape(B, P)
    ltype = np.asarray(ligand_atom_type).reshape(B, L)

    # fused stage-1 weight: z1 = W1z.T @ [rb; onehot_p; onehot_l]
    PA = prot_emb @ W1[0:H, :]
    LA = lig_emb @ W1[H:2 * H, :] + b1[None, :]
    W1z = np.concatenate([W1[2 * H:2 * H + RB, :], PA, LA],
                         axis=0).astype(np.float16)
    wbb = np.concatenate([W2, W3], axis=1).astype(np.float16)
    fbb = np.ascontiguousarray(
        np.stack([b2.reshape(H), b3.reshape(H)], axis=1).astype(np.float32))

    centers = np.linspace(0.0, CUTOFF, RB, dtype=np.float32)

    # per-complex valid pairs -> global list of 512-pair tiles
    full_tiles = []     # (complex, rhs [K1,512] fp32) with no pad columns
    padded_tiles = []   # each complex's final, partially-padded tile
    cnts = []
    for b in range(B):
        diff = protein_pos[b][:, None, :] - ligand_pos[b][None, :, :]
        dist = np.sqrt((diff * diff).sum(-1, dtype=np.float32))
        pi, li = np.nonzero(dist < np.float32(CUTOFF))
        cnt = len(pi)
        cnts.append(cnt)
        dv = dist[pi, li]
        rhs = np.zeros((K1, ((cnt + 511) // 512) * 512), dtype=np.float32)
        rhs[0:RB, :cnt] = np.exp(
            -0.5 * ((dv[:, None] - centers[None, :]) / WIDTH) ** 2).T
        rhs[RB + ptype[b][pi], np.arange(cnt)] = 1.0
        rhs[RB + NPT + ltype[b][li], np.arange(cnt)] = 1.0
        nt_b = rhs.shape[1] // 512
        for s in range(nt_b):
            t = (b, rhs[:, 512 * s:512 * (s + 1)])
            if s == nt_b - 1 and cnt % 512 != 0:
                padded_tiles.append(t)
            else:
                full_tiles.append(t)

    ntot = len(full_tiles) + len(padded_tiles)
    nt = max(NT_DEFAULT, (ntot + N_CORES - 1) // N_CORES)
    # every core's LAST slot must contain a pad at column 511 (cvec source):
    # full tiles round-robin first, then one padded tile per core at the end;
    # any remaining slots become all-pad filler tiles.
    while True:
        core_tiles = [[] for _ in range(N_CORES)]
        core_pad = [False] * N_CORES
        for i, tb in enumerate(full_tiles):
            core_tiles[i % N_CORES].append(tb)
        for i, tb in enumerate(padded_tiles):
            core_tiles[i % N_CORES].append(tb)
            core_pad[i % N_CORES] = True
        ok = all(len(ct) <= nt and (len(ct) < nt or core_pad[k])
                 for k, ct in enumerate(core_tiles))
        if ok:
            break
        nt += 1
    npair = nt * 512

    in_maps = []
    tilemap = []                 # per core: complex id per tile (-1 = pad)
    for k in range(N_CORES):
        rhs = np.zeros((K1, npair), dtype=np.float32)
        cmap = []
        for s, (b, rt) in enumerate(core_tiles[k]):
            rhs[:, 512 * s:512 * (s + 1)] = rt
            cmap.append(b)
        cmap += [-1] * (nt - len(cmap))
        tilemap.append(cmap)
        rhs16 = rhs.astype(np.float16)
        wa = np.concatenate([W1z, rhs16[:, 0:512]], axis=1)
        in_maps.append({"rhsG": rhs16, "wa": np.ascontiguousarray(wa),
                        "wb": wbb, "fb": fbb})

    nc = _get_nc(nt)
    res = bass_utils.run_bass_kernel_spmd(nc, in_maps,
                                          core_ids=list(range(N_CORES)))

    # unshard: attribute tile sums to complexes, remove pad contributions,
    # mean + tiny fp32 scoring head
    cvec = res.results[0]["cvec"][:, 0]
    tot = np.zeros((B, H), dtype=np.float64)
    ntiles_b = np.zeros(B, dtype=np.int64)
    for k in range(N_CORES):
        acc = np.concatenate([res.results[k]["accA"],
                              res.results[k]["accB"]], axis=1)
        for s, b in enumerate(tilemap[k]):
            if b >= 0:
                tot[b] += acc[:, s]
                ntiles_b[b] += 1
    out = np.zeros(B, dtype=np.float32)
    for b in range(B):
        cnt = cnts[b]
        if cnt == 0:
            continue
        npad = ntiles_b[b] * 512 - cnt
        t2 = (tot[b] - npad * cvec).astype(np.float32)
        repr_ = (t2 / cnt).astype(np.float32)
        r1 = np.maximum(repr_ @ Wr1 + br1, 0.0)
        out[b] = r1 @ Wr2 + br2
    return out
